# revision 73
# baseline (speedup 1.0000x reference)
# Trainium2 Bass kernel for a causal multi-head attention block.
#
# Reference computation (fp32):
#   qkv = x @ w_attn + b_attn ; split into q,k,v heads (N=16, H=64)
#   scores = q @ k^T / sqrt(H), causal mask, softmax over keys
#   out = (weights @ v) reshaped, then out @ w_proj + b_proj
#
# Sharding: 8 cores = 2 batches x 4 head-groups (4 heads each).
#   - batch data-parallel, heads tensor-parallel (c_attn columns / c_proj rows)
#   - each core emits a partial [T, D] projection output (bf16); host sums the
#     4 head-group partials per batch in f32 and adds b_proj (the gather step).
#
# v3 design notes:
#   - full bf16 datapath (psum accumulation stays f32): every matmul runs at
#     1 cycle/row for any free-dim.
#   - x^T is produced on the HOST (numpy transpose) and loaded with a handful
#     of large strided DMAs: DMAs sit on serialized DGE rings with ~2.6us of
#     completion-chain latency each, so minimizing DMA instruction count is
#     critical for the front-end.
#   - attn^T for the projection uses cheap bf16 PE transposes (1 cycle/row)
#     into a small psum scratch, keeping the projection off the DMA rings.
#   - scores are computed TRANSPOSED (S^T[s,t]) so exp(S^T) tiles feed the
#     weights@V matmul directly as the stationary operand; row sums come free
#     via ones-columns in V; softmax normalize is a single fused tensor_scalar
#     divide (psum -> sbuf) per 128x64 block.

import math

import numpy as np

B, T, D = 2, 2048, 1024
NHEAD, H = 16, 64
HPC = 4            # heads per core
CD = HPC * H       # 256 head-dim columns per core
N_CORES = 8
P = 128            # partitions
TB = T // 512      # 4 t-blocks of 512
KD = D // P        # 8 contraction tiles over D
G = H + 1          # AV output cols per head (64 + ones-col for rowsum)

_CACHE = {}


def _build_module():
    import contextlib

    import concourse.bass as bass  # noqa: F401
    import concourse.mybir as mybir
    import concourse.tile as tile
    from concourse import bacc

    f32 = mybir.dt.float32
    bf = mybir.dt.bfloat16

    nc = bacc.Bacc("TRN2", target_bir_lowering=False, debug=False)

    xT_d = nc.dram_tensor("xT", [D, T], bf, kind="ExternalInput").ap()
    wqkv_d = nc.dram_tensor("wqkv", [D, 3 * CD], bf, kind="ExternalInput").ap()
    bqk_d = nc.dram_tensor("bqk", [P, 4], f32, kind="ExternalInput").ap()
    bv_d = nc.dram_tensor("bv", [P, CD], f32, kind="ExternalInput").ap()
    wp_d = nc.dram_tensor("wp", [CD, D], bf, kind="ExternalInput").ap()
    ident_d = nc.dram_tensor("ident", [P, P], bf, kind="ExternalInput").ap()
    mask_d = nc.dram_tensor("mask", [P, P], bf, kind="ExternalInput").ap()
    ones_d = nc.dram_tensor("onescol", [P, HPC], bf, kind="ExternalInput").ap()
    y_d = nc.dram_tensor("y", [T, D], bf, kind="ExternalOutput").ap()

    with tile.TileContext(nc) as tc, contextlib.ExitStack() as ctx:
        const_p = ctx.enter_context(tc.tile_pool(name="const", bufs=1))
        w_p = ctx.enter_context(tc.tile_pool(name="weights", bufs=1))
        xt_p = ctx.enter_context(tc.tile_pool(name="xt", bufs=1))
        qkt_p = ctx.enter_context(tc.tile_pool(name="qkt", bufs=1))
        v_p = ctx.enter_context(tc.tile_pool(name="vbuf", bufs=1))
        e_p = ctx.enter_context(tc.tile_pool(name="epool", bufs=16))
        attn_p = ctx.enter_context(tc.tile_pool(name="attn", bufs=1))
        at_p = ctx.enter_context(tc.tile_pool(name="attnT", bufs=1))
        ysb_p = ctx.enter_context(tc.tile_pool(name="ysb", bufs=4))
        small_p = ctx.enter_context(tc.tile_pool(name="small", bufs=8))
        # single PSUM pool, 8 banks total:
        #   wps  [128,512]f32 x2  (phase1 qk/V accum + proj jb<3)   2 banks
        #   sp   [128,1024]f32 x2 (scores, 2 heads side by side)    4 banks
        #   accp [128,264]f32 x2  (AV accumulators, 4 groups each)  2 banks
        #   tp   [128,512]bf16 x1 (attn^T transposes)               in slack
        psp = ctx.enter_context(tc.tile_pool(name="psp", bufs=2, space="PSUM"))

        # ---- PE warmup ----
        # ~3us of junk matmuls on a memset tile during the initial DMA
        # latency window: the tensor engine's p-state ramp needs >3us of
        # continuous busy before matmuls run at full clock, so the ramp is
        # paid on junk instead of the first real accumulation chains.
        junk = const_p.tile([P, 5 * P], bf, name="junk_sb")
        nc.gpsimd.memset(junk, 0)
        jps = psp.tile([P, 1024], f32, name="warm", tag="sp")
        for _ in range(8):
            nc.tensor.matmul(jps[:, 0:512], junk[:, 0:P], junk[:, P:],
                             start=True, stop=True)

        # ---- loads ----
        # A handful of big strided DMAs: x^T was pre-transposed on the host,
        # so its k-chunks land as plain copies. DMAs chain serially on the
        # DGE rings / DMA bus, so the host reorders wqkv columns to
        # [q0|k0|v|q1|k1] and the loads are split so the bytes the first
        # attention steps need arrive first.
        wqkv_sb = w_p.tile([P, KD * 3 * CD], bf, name="wqkv_sb")
        wqkv_v = wqkv_sb.rearrange("p (k c) -> p k c", k=KD)
        xt_sb = xt_p.tile([P, KD * T], bf, name="xt_sb")
        xt_v = xt_sb.rearrange("p (k t) -> p k t", k=KD)
        MOFF = {0: 0, 2: P, 1: 4 * P, 3: 5 * P}  # m-group -> host column base
        VOFF = 2 * P  # v columns 256:512

        def load_wqkv_cols(c0, c1):
            nc.sync.dma_start(
                wqkv_v[:, :, c0:c1],
                wqkv_d[:, c0:c1].rearrange("(k p) c -> p k c", p=P),
            )

        def load_xt(k0, k1, t0, t1):
            nc.sync.dma_start(
                xt_v[:, k0:k1, t0:t1],
                xT_d[P * k0 : P * k1, t0:t1].rearrange("(k p) t -> p k t", p=P),
            )

        def load_wqkv_k(k0, k1, c0, c1):
            nc.sync.dma_start(
                wqkv_v[:, k0:k1, c0:c1],
                wqkv_d[P * k0 : P * k1, c0:c1].rearrange("(k p) c -> p k c", p=P),
            )

        load_wqkv_k(0, 4, 0, 2 * P)     # q0|k0 first k-half
        load_xt(0, 4, 0, 512)
        load_wqkv_k(4, 8, 0, 2 * P)     # q0|k0 second k-half
        load_xt(4, 8, 0, 512)
        load_wqkv_cols(2 * P, 4 * P)    # v
        load_wqkv_cols(4 * P, 6 * P)    # q1|k1
        load_xt(0, 8, 512, 1024)
        load_xt(0, 8, 1024, 2048)

        def xts(k, j):
            return xt_v[:, k, 512 * j : 512 * (j + 1)]

        # consts + wp on the ACT hwdge ring (ACT is idle at kernel start)
        bqk = const_p.tile([P, 4], f32, name="bqk_sb")
        nc.scalar.dma_start(bqk, bqk_d)
        ident = const_p.tile([P, P], bf, name="ident_sb")
        nc.scalar.dma_start(ident, ident_d)
        mask = const_p.tile([P, P], bf, name="mask_sb")
        nc.scalar.dma_start(mask, mask_d)
        onescol = const_p.tile([P, HPC], bf, name="ones_sb")
        nc.scalar.dma_start(onescol, ones_d)
        bv = const_p.tile([P, CD], f32, name="bv_sb")
        nc.scalar.dma_start(bv, bv_d)
        wp_sb = []
        for c in range(CD // P):
            w = w_p.tile([P, D], bf, name=f"wp{c}", tag=f"wp{c}")
            nc.scalar.dma_start(w, wp_d[P * c : P * (c + 1), :])
            wp_sb.append(w)

        # persistent activation buffers
        qkt_sb = {}
        for m in range(4):
            for j in range(TB):
                qkt_sb[(m, j)] = qkt_p.tile(
                    [P, 512], bf, name=f"qkt{m}_{j}", tag=f"qkt{m}_{j}"
                )
        v_sb = []
        for i in range(T // P):
            v_sb.append(v_p.tile([P, HPC * G], bf, name=f"v{i}", tag=f"v{i}"))
        attn_t = {
            (tb, hp): attn_p.tile([P, 512], bf, name=f"attn{tb}_{hp}",
                                  tag=f"attn{tb}_{hp}")
            for tb in range(TB)
            for hp in range(2)
        }
        attnT = {
            (tb, hp): at_p.tile([P, 512], bf, name=f"at{tb}_{hp}",
                                tag=f"at{tb}_{hp}")
            for tb in range(TB)
            for hp in range(2)
        }

        # ones-columns in V are static: set them once upfront on gpsimd
        for i in range(T // P):
            nc.gpsimd.tensor_copy(
                v_sb[i].rearrange("p (g c) -> p g c", g=HPC)[:, :, H:G],
                onescol.rearrange("p (g c) -> p g c", c=1),
            )

        def qk_group(m, j, chunk=KD):
            """Generator: emits the qk chain for (m, j) in `chunk`-MM slices
            so the caller can interleave attention score matmuls between
            slices (avoids priority inversion starving the ACT exp stream)."""
            ps = psp.tile([P, 512], f32, name="qkp", tag="wps")
            for k in range(KD):
                nc.tensor.matmul(
                    ps,
                    wqkv_v[:, k, MOFF[m] : MOFF[m] + P],
                    xts(k, j),
                    start=(k == 0),
                    stop=(k == KD - 1),
                )
                if k % chunk == chunk - 1 and k < KD - 1:
                    yield
            # psum->sbuf drain with the per-partition qk bias folded in
            nc.vector.tensor_scalar_add(qkt_sb[(m, j)], ps, bqk[:, m : m + 1])

        def v_group(g, chunk=KD):
            j, ti = g // 4, g % 4
            ps = psp.tile([P, 512], f32, name="vp", tag="wps")
            for k in range(KD):
                nc.tensor.matmul(
                    ps[:, 0:CD],
                    xts(k, j)[:, P * ti : P * (ti + 1)],
                    wqkv_v[:, k, VOFF : VOFF + CD],
                    start=(k == 0),
                    stop=(k == KD - 1),
                )
                if k % chunk == chunk - 1 and k < KD - 1:
                    yield
            # psum->sbuf drain with the (partition-broadcast) V bias
            vg = v_sb[g].rearrange("p (g c) -> p g c", g=HPC)
            nc.vector.tensor_add(
                vg[:, :, 0:H],
                ps[:, 0:CD].rearrange("p (g c) -> p g c", g=HPC),
                bv.rearrange("p (g c) -> p g c", g=HPC),
            )

        def run_all(gen):
            for _ in gen:
                pass



        def attention_hp(tb, hp, step_cb=None):
            """S^T -> exp -> AV for one head pair of 512-wide t-block tb
            (partition bases 0 and 64); one psum tile holds both heads'
            scores so a single exp covers both. `step_cb(i)` lets the caller
            inject PE filler work right behind step i's score matmuls (the
            phase-1 chains are pumped here in small slices so they never
            starve the ACT exp stream)."""
            if True:
                mq, mk = hp, 2 + hp
                acc_t = [
                    psp.tile([P, 4 * G], f32, name="accp", tag=f"accp{a}", bufs=1)
                    for a in range(2)
                ]
                n_s = 4 * tb + 4  # s-tiles 0 .. 4*tb+3
                for i in range(n_s):
                    first = max(0, i - 4 * tb)  # first valid jj in block
                    c0 = P * first
                    sps = psp.tile([P, 1024], f32, name="sp", tag="sp", bufs=2)
                    for hh, pb in ((0, 0), (1, 64)):
                        nc.tensor.matmul(
                            sps[:, 512 * hh + c0 : 512 * hh + 512],
                            qkt_sb[(mk, i // 4)][
                                pb : pb + H, P * (i % 4) : P * (i % 4 + 1)
                            ],
                            qkt_sb[(mq, tb)][pb : pb + H, c0:512],
                            start=True,
                            stop=True,
                        )
                    if step_cb is not None:
                        step_cb(i)
                    et = e_p.tile([P, 1024], bf, name="et", tag="et")
                    if first:
                        nc.scalar.activation(
                            et.rearrange("p (g c) -> p g c", g=2)[:, :, c0:512],
                            sps.rearrange("p (g c) -> p g c", g=2)[:, :, c0:512],
                            mybir.ActivationFunctionType.Exp,
                            scale=1.0 / math.sqrt(H),
                        )
                    else:
                        nc.scalar.activation(
                            et,
                            sps,
                            mybir.ActivationFunctionType.Exp,
                            scale=1.0 / math.sqrt(H),
                        )
                    dj = i - 4 * tb  # diagonal jj of this s-tile, if any
                    etd = None
                    if 0 <= dj <= 3:
                        # masked diagonal sub-tiles go to a separate tile so
                        # the non-diagonal AV matmuls don't serialize behind
                        # the mask write (tile-granular dependency tracking)
                        etd = e_p.tile([P, 2 * P], bf, name="etd", tag="etd",
                                       bufs=2)
                        for hh in range(2):
                            nc.vector.tensor_mul(
                                etd[:, P * hh : P * (hh + 1)],
                                et[:, 512 * hh + P * dj : 512 * hh + P * (dj + 1)],
                                mask,
                            )
                    for jj in range(first, 4):
                        jglob = 4 * tb + jj
                        for hh in range(2):
                            if jj == dj:
                                lhs_e = etd[:, P * hh : P * (hh + 1)]
                            else:
                                lhs_e = et[
                                    :, 512 * hh + P * jj : 512 * hh + P * (jj + 1)
                                ]
                            # start=True clears has_written for the WHOLE
                            # psum bank: only the first group per bank
                            # issues it.
                            nc.tensor.matmul(
                                acc_t[hh][:, G * jj : G * jj + G],
                                lhs_e,
                                v_sb[i][:, G * (2 * hp + hh) : G * (2 * hp + hh) + G],
                                start=(i == 0 and jj == 0),
                                stop=(i == jglob),
                                skip_group_check=True,
                            )
                    if 0 <= dj <= 3:
                        # acc group dj just received its last (diagonal)
                        # contribution: normalize it now so the psum bank
                        # region drains while later s-tiles still accumulate
                        for hh in range(2):
                            s0 = G * dj
                            rec = small_p.tile([P, 1], f32, name="rec", tag="rec")
                            nc.vector.reciprocal(
                                rec, acc_t[hh][:, s0 + H : s0 + H + 1]
                            )
                            nc.vector.tensor_scalar_mul(
                                attn_t[(tb, hp)][
                                    :, P * dj + H * hh : P * dj + H * (hh + 1)
                                ],
                                acc_t[hh][:, s0 : s0 + H],
                                rec,
                            )
                # attn^T for the projection: bf16 PE transposes (1 cycle/row)
                # reusing this head pair's just-drained AV accumulator bank
                # (bitcast f32->bf16 view): the ring dependency is exactly
                # "this hp's normalizes are done", so neither the next score
                # matmul nor phase 1 is ever serialized behind the transpose.
                pt = psp.tile([P, 4 * G], f32, name="atp", tag="accp0",
                              bufs=1).bitcast(bf)
                for dj in range(4):
                    nc.tensor.transpose(
                        pt[:, P * dj : P * (dj + 1)],
                        attn_t[(tb, hp)][:, P * dj : P * (dj + 1)],
                        ident,
                    )
                nc.vector.tensor_copy(attnT[(tb, hp)], pt[:, 0:512])

        def projection(jb):
            """y = attn @ wp for 512-wide t-block jb."""
            for jl in range(4):
                jt = 4 * jb + jl
                ysb = ysb_p.tile([P, D], bf, name="ysb", tag="ysb")
                # proj(3) spreads over BOTH psum rings (all free by then) so
                # four chains are in flight and the tail isn't ring-paced
                merged = jb >= 2 and not (jb == 3 and jl == 3)
                if merged:
                    pss = psp.tile([P, 1024], f32, name="yp", tag="sp")
                for n in range(2):
                    if merged:
                        ps = pss[:, 512 * n : 512 * (n + 1)]
                    else:
                        ps = psp.tile([P, 512], f32, name="yp", tag="wps")
                    for hp in range(2):
                        nc.tensor.matmul(
                            ps,
                            attnT[(jb, hp)][:, P * jl : P * (jl + 1)],
                            wp_sb[hp][:, 512 * n : 512 * (n + 1)],
                            start=(hp == 0),
                            stop=(hp == 1),
                        )
                    if jb == 3 and jl == 3:
                        # last tile: half-drain + half-store per engine/ring
                        if n == 0:
                            nc.scalar.copy(ysb[:, 0:512], ps)
                            nc.scalar.dma_start(
                                y_d[P * jt : P * (jt + 1), 0:512], ysb[:, 0:512]
                            )
                        else:
                            nc.vector.tensor_copy(ysb[:, 512:1024], ps)
                            nc.sync.dma_start(
                                y_d[P * jt : P * (jt + 1), 512:1024],
                                ysb[:, 512:1024],
                            )
                    elif not merged:
                        if jb >= 2 and (jt + n) % 2 == 1:
                            # near/after the end of the exp stream: split
                            # drains across DVE/ACT
                            nc.scalar.copy(ysb[:, 512 * n : 512 * (n + 1)], ps)
                        else:
                            # mid-kernel drains stay off ACT (exp stream live)
                            nc.vector.tensor_copy(
                                ysb[:, 512 * n : 512 * (n + 1)], ps
                            )
                if merged:
                    # merged drains alternate DVE/ACT (exp stream is done or
                    # finishing by the time proj(2)/proj(3) drain)
                    if jl % 2 == 0:
                        nc.vector.tensor_copy(ysb, pss)
                    else:
                        nc.scalar.copy(ysb, pss)
                if jb == 3 and jl == 3:
                    # the very last tile's drains/stores went out per-half on
                    # separate engines and DGE rings above: nothing to do
                    continue
                # last block's stores on the HWDGE ring (no SWDGE gen latency
                # on the kernel tail)
                (nc.sync if jb == 3 else nc.gpsimd).dma_start(
                    y_d[P * jt : P * (jt + 1), :], ysb
                )

        # emission order: each block's head-pair-0 attention starts as soon
        # as its q0/k0 groups exist, with the block's V chains injected into
        # the exp latency of the first attention steps; later phase-1 work
        # fills attention's (ACT-bound) PE slack via the scheduler;
        # projections run at the end where psum banks are free.
        from collections import deque

        fill = deque()

        def pump(n):
            done = 0
            while fill and done < n:
                try:
                    next(fill[0])
                except StopIteration:
                    fill.popleft()
                done += 1

        def flush():
            while fill:
                pump(99)

        # Block 0 is front-latency-bound (ACT has nothing queued yet), so its
        # phase-1 chains are emitted coarsely; blocks 1-3 pump their phase-1
        # chains in 2-MM slices between attention steps so pending score
        # matmuls are never delayed by more than one slice.
        run_all(qk_group(0, 0))
        run_all(qk_group(2, 0))

        def cb0(i):
            run_all(v_group(i))
            if i == 2:
                run_all(qk_group(1, 0))
            elif i == 3:
                run_all(qk_group(3, 0))

        attention_hp(0, 0, step_cb=cb0)
        fill.append(qk_group(0, 1, chunk=2))
        fill.append(qk_group(2, 1, chunk=2))
        attention_hp(0, 1, step_cb=lambda i: pump(2))
        for j in (1, 2, 3):
            flush()
            for ti in range(4):
                fill.append(v_group(4 * j + ti, chunk=2))
            fill.append(qk_group(1, j, chunk=2))
            fill.append(qk_group(3, j, chunk=2))
            attention_hp(j, 0, step_cb=lambda i, j=j: pump(3 if j == 1 else 2))
            flush()
            if j < 3:
                fill.append(qk_group(0, j + 1, chunk=2))
                fill.append(qk_group(2, j + 1, chunk=2))
            attention_hp(j, 1, step_cb=lambda i: pump(1))
        flush()
        projection(0)
        projection(1)
        projection(2)
        projection(3)

    nc.compile()
    return nc


def _get_module():
    if "m" not in _CACHE:
        _CACHE["m"] = _build_module()
    return _CACHE["m"]


def kernel(x, w_attn, b_attn, w_proj, b_proj, **_ignored):
    import ml_dtypes
    from concourse.bass_utils import run_bass_kernel_spmd

    bfnp = np.dtype(ml_dtypes.bfloat16)
    x = np.asarray(x, dtype=np.float32)
    w_attn = np.asarray(w_attn, dtype=np.float32)
    b_attn = np.asarray(b_attn, dtype=np.float32)
    w_proj = np.asarray(w_proj, dtype=np.float32)
    b_proj = np.asarray(b_proj, dtype=np.float32)

    nc = _get_module()

    mask = np.triu(np.ones((P, P), dtype=bfnp))
    ident = np.eye(P, dtype=bfnp)
    xT = [np.ascontiguousarray(x[b].T).astype(bfnp) for b in range(B)]

    in_maps = []
    for core in range(N_CORES):
        b = core // 4
        g = core % 4
        c0 = CD * g
        wq = w_attn[:, c0 : c0 + CD]
        wk = w_attn[:, D + c0 : D + c0 + CD]
        wv = w_attn[:, 2 * D + c0 : 2 * D + c0 + CD]
        bq = b_attn[c0 : c0 + CD]
        bk = b_attn[D + c0 : D + c0 + CD]
        bvv = b_attn[2 * D + c0 : 2 * D + c0 + CD]
        in_maps.append(
            {
                "xT": xT[b],
                # column order [q0|k0|v|q1|k1]: the first attention steps
                # only need q0/k0, so those bytes ship first
                "wqkv": np.concatenate(
                    [wq[:, 0:P], wk[:, 0:P], wv, wq[:, P:CD], wk[:, P:CD]], axis=1
                ).astype(bfnp),
                "bqk": np.concatenate([bq, bk]).reshape(4, P).T.copy(),
                "bv": np.broadcast_to(bvv[None, :], (P, CD)).copy(),
                "wp": np.ascontiguousarray(w_proj[c0 : c0 + CD, :]).astype(bfnp),
                "ident": ident,
                "mask": mask,
                "onescol": np.ones((P, HPC), dtype=bfnp),
            }
        )

    res = run_bass_kernel_spmd(nc, in_maps, core_ids=list(range(N_CORES)))

    out = np.zeros((B, T, D), dtype=np.float32)
    for core in range(N_CORES):
        out[core // 4] += res.results[core]["y"].astype(np.float32)
    out += b_proj[None, None, :]
    return out
